# revision 23
# baseline (speedup 1.0000x reference)
"""AttentionBlock Trainium2 kernel (v3d).

Sharding: data-parallel over batch (B=8 -> one batch element per NeuronCore).

Per-core pipeline (C=512, HW=1024, 8 heads x 64):
  groupnorm (stats via indicator matmuls, f32), per-half chains so ht[0]
  is ready before all x tiles land
  -> h cast to fp8 pair-tile layout [128,2,1024] (c = 256t + 128j + p)
  -> QK GEMM fp8 DR, one head per 128-row block; q-block rows:
     q(0:64), tq=bk.Wq.h(64), zeros(65:128) -- the GEMM itself writes the
     zero padding that keeps HAM at full rate (66-row contraction flips
     the PE into the k=4 half-rate throttle). k-block: k(0:64), zeros.
     tk rows ride an 8-row aug block, gather-DMA'd to KsB row 65 once.
  -> Q,K evacuated to bf16 scores operands [128, NH, 1024] by aligned
     copies (no bulk SBUF-SBUF shuffles: on-chip DMA is only ~20 GB/s);
     ones rows (Q:65, K:64) via two small early DMAs; rows 66:128 zeroed
     once by GpSimd memsets. Exp bias is then a per-head CONSTANT.
  -> scores GEMM bf16 [k,q], full 128-row contraction; exp on ScalarE,
     bias SCALE*bq.bk - OFF; probs fp8 (max 240)
  -> AV GEMM fp8 DR over kk-pairs; denominators ride as a ones-column
  -> normalize: PE ones-broadcast of denom row + reciprocal_approx_fast,
     multiply on GpSimd; EVEN heads partition-shifted into attO[64:128]
     by small on-chip DMA (odd heads -- incl. the last -- write direct)
     -> attO fp8 [128,2,1024] x2, densely packed
  -> proj GEMM fp8 DR, 2 contraction passes, + residual
     (+ W_p b_v + b_proj folded per-channel).
"""
import sys

sys.path.insert(0, "/opt/trn_rl_repo")
import numpy as np
import ml_dtypes
import concourse.bass as bass
import concourse.bacc as bacc
import concourse.tile as tile
from concourse import mybir
from concourse.bass_utils import run_bass_kernel_spmd

f32 = mybir.dt.float32
f32r = mybir.dt.float32r
bf16 = mybir.dt.bfloat16
fp8 = mybir.dt.float8e4
ALU = mybir.AluOpType
ACT = mybir.ActivationFunctionType
DR = mybir.MatmulPerfMode.DoubleRow

C = 512
HW = 1024
NH = 8
HD = 64
EPS = 1e-5
SCALE = HD ** -0.5
NT = C // 128  # 4 channel tiles
NP = HW // 128  # 8 position tiles

OFF = 2.0          # exp offset: probs = exp(s*SCALE - OFF)
A8 = 8.0 / np.log(2.0)      # Schraudolph slope for fp8e4m3 bit pattern
B8 = 7 * 8                  # fp8e4m3 exponent bias << 3
CC8 = 0.45                  # Schraudolph bias correction (min-RMS)
# kk tiles whose exp runs on DVE (Schraudolph) instead of ScalarE
EXP_DVE_KK = (1, 4, 6)


def _build():
    nc = bacc.Bacc("TRN2", target_bir_lowering=False, debug=False, num_devices=8)
    x_d = nc.dram_tensor("x", [C, HW], f32, kind="ExternalInput").ap()
    wqk_d = nc.dram_tensor("wqk", [128, 2, 2, 2112], fp8, kind="ExternalInput").ap()
    wv_d = nc.dram_tensor("wv8", [128, 2, 2, 512], fp8, kind="ExternalInput").ap()
    wp_d = nc.dram_tensor("wp8", [128, 2, 2, 512], fp8, kind="ExternalInput").ap()
    wtld_d = nc.dram_tensor("wtld", [C], f32, kind="ExternalInput").ap()
    gamma_d = nc.dram_tensor("gamma", [C], f32, kind="ExternalInput").ap()
    beta_d = nc.dram_tensor("beta", [C], f32, kind="ExternalInput").ap()
    gind_d = nc.dram_tensor("gind", [128, 8], f32, kind="ExternalInput").ap()
    gbc_d = nc.dram_tensor("gbc", [8, 128], f32, kind="ExternalInput").ap()
    cb8_d = nc.dram_tensor("cb8", [8], f32, kind="ExternalInput").ap()
    cs8_d = nc.dram_tensor("cs8", [8], f32, kind="ExternalInput").ap()
    onesb_d = nc.dram_tensor("onesb", [1, 8192], bf16, kind="ExternalInput").ap()
    cnsb_d = nc.dram_tensor("cnsb", [2, 1024], bf16, kind="ExternalInput").ap()
    out_d = nc.dram_tensor("out", [C, HW], f32, kind="ExternalOutput").ap()

    with tile.TileContext(nc) as tc:
        with (
            tc.tile_pool(name="const", bufs=1) as cp,
            tc.tile_pool(name="gnp", bufs=2) as gnp,
            tc.tile_pool(name="xp", bufs=1) as xp,
            tc.tile_pool(name="hp", bufs=1) as hp,
            tc.tile_pool(name="qs", bufs=1) as qsp,
            tc.tile_pool(name="vt", bufs=1) as vtp,
            tc.tile_pool(name="pp", bufs=8) as ppp,
            tc.tile_pool(name="pbf", bufs=2) as pbfp,
            tc.tile_pool(name="dsb", bufs=3) as dsbp,
            tc.tile_pool(name="rb", bufs=2) as rbp,
            tc.tile_pool(name="ao", bufs=1) as aop,
            tc.tile_pool(name="psA", bufs=2, space="PSUM") as psA,
            tc.tile_pool(name="psB", bufs=2, space="PSUM") as psB,
        ):
            # ---------------- loads ----------------
            wqk = cp.tile([128, 2, 2, 2112], fp8, tag="wqk")
            wv = cp.tile([128, 2, 2, 512], fp8, tag="wv")
            wp = cp.tile([128, 2, 2, 512], fp8, tag="wp")

            def col_load(src_ap, offset, name):
                t_ = cp.tile([128, NT], f32, tag=name)
                nc.sync.dma_start(
                    out=t_,
                    in_=bass.AP(tensor=src_ap.tensor, offset=offset,
                                ap=[[1, 128], [128, NT]]),
                )
                return t_

            gamma_sb = col_load(gamma_d, 0, "gamma")
            beta_sb = col_load(beta_d, 0, "beta")
            wtld_sb = col_load(wtld_d, 0, "wtld")

            def bcast_load(src_ap, n, name):
                t_ = cp.tile([128, n], f32, tag=name)
                nc.sync.dma_start(
                    out=t_,
                    in_=bass.AP(tensor=src_ap.tensor, offset=0,
                                ap=[[0, 128], [1, n]]))
                return t_

            cb8_sb = bcast_load(cb8_d, 8, "cb8")
            cs8_sb = bcast_load(cs8_d, 8, "cs8")

            o64b = cp.tile([65, 64], bf16, tag="o64b")
            nc.sync.dma_start(out=o64b[64:65, :], in_=cnsb_d[0:1, 0:64])
            gind_f = cp.tile([128, 8], f32, tag="gindf")
            nc.sync.dma_start(out=gind_f, in_=gind_d)
            gbc_f = cp.tile([8, 128], f32, tag="gbcf")
            nc.sync.dma_start(out=gbc_f, in_=gbc_d)

            xt = []
            for t in range(NT):
                xi = xp.tile([128, HW], f32, tag=f"x{t}")
                nc.sync.dma_start(out=xi, in_=x_d[t * 128:(t + 1) * 128, :])
                xt.append(xi)
                if t < 2:
                    nc.sync.dma_start(out=wqk[:, t], in_=wqk_d[:, t])
            nc.sync.dma_start(out=wv, in_=wv_d)
            nc.scalar.dma_start(out=wp, in_=wp_d)

            # Q/K bf16 scores operands [128, NH, HW], parity-split rows:
            # even head h: q(0:64), tq(64), ones(65), zeros(66:128)
            #              [KsB: k(0:64), ones(64), tk(65)]
            # odd head h:  tq(0), ones(1), zeros(2:64), q(64:128)
            #              [KsB: ones(0), tk(1), k(64:128)]
            # so both halves of a 2-head QK GEMM block evacuate
            # partition-aligned. tq/tk rows ride a 64-row aug W block.
            QsB = qsp.tile([128, NH, HW], bf16, tag="QsB")
            KsB = qsp.tile([128, NH, HW], bf16, tag="KsB")
            QsBv = QsB.rearrange("p (b s) f -> p s b f", s=2)
            KsBv = KsB.rearrange("p (b s) f -> p s b f", s=2)
            nc.gpsimd.memset(QsBv[64:128, 0], 0.0)
            nc.gpsimd.memset(QsBv[0:64, 1], 0.0)
            nc.gpsimd.memset(KsBv[64:128, 0], 0.0)
            nc.gpsimd.memset(KsBv[0:64, 1], 0.0)
            nc.sync.dma_start(out=QsBv[65:66, 0], in_=onesb_d[0:1, 0:4096])
            nc.sync.dma_start(out=QsBv[1:2, 1], in_=onesb_d[0:1, 0:4096])
            nc.sync.dma_start(out=KsBv[64:65, 0], in_=onesb_d[0:1, 0:4096])
            nc.sync.dma_start(out=KsBv[0:1, 1], in_=onesb_d[0:1, 0:4096])

            # vT tiles per kk-pair: [128, 2, NH, 96-padded]; col 64 = ones.
            ones16f = cp.tile([128, 16], f32, tag="ones16f")
            nc.gpsimd.memset(ones16f, 1.0)
            vt = []
            for kp in range(4):
                v = vtp.tile([128, 2, NH, 96], fp8, tag=f"vt{kp}")
                nc.vector.tensor_copy(
                    out=v[:, :, :, HD:HD + 1].rearrange("p a b c -> p (a b c)"),
                    in_=ones16f)
                vt.append(v)

            # ---------------- groupnorm (two per-half chains) ----------------
            eps_t = cp.tile([128, 1], f32, tag="eps")
            nc.vector.memset(eps_t, EPS)
            gind_r = cp.tile([128, 8], f32r, tag="gindr")
            nc.vector.tensor_copy(out=gind_r, in_=gind_f)
            gbc_r = cp.tile([8, 128], f32r, tag="gbcr")
            nc.vector.tensor_copy(out=gbc_r, in_=gbc_f)

            ht = [hp.tile([128, 2, HW], fp8, tag=f"h{t}", name=f"h{t}")
                  for t in range(2)]
            pg = psB.tile([128, 1024], f32, tag="B", name="pg")
            pgv = pg[0:8, 0:8].rearrange("p (a b) -> p a b", b=2)
            for half in range(2):
                for t in (2 * half, 2 * half + 1):
                    st = gnp.tile([128, 2, 6], f32, tag="bnst")
                    nc.vector.bn_stats(out=st[:, 0, :], in_=xt[t][:, 0:512])
                    nc.vector.bn_stats(out=st[:, 1, :], in_=xt[t][:, 512:1024])
                    mv = gnp.tile([128, 2], f32, tag="mv")
                    nc.vector.bn_aggr(out=mv, in_=st)
                    me = gnp.tile([128, 2], f32r, tag=f"me{t}", name=f"me{t}")
                    nc.vector.tensor_copy(out=me[:, 0:1], in_=mv[:, 0:1])
                    sq = gnp.tile([128, 1], f32, tag="sq")
                    nc.vector.tensor_mul(out=sq, in0=mv[:, 0:1], in1=mv[:, 0:1])
                    nc.vector.tensor_add(out=me[:, 1:2], in0=mv[:, 1:2], in1=sq)
                    nc.tensor.matmul(
                        pgv[:, t, :], lhsT=gind_r, rhs=me,
                        start=True, stop=True)
                sl = slice(2 * half, 2 * half + 2)
                mE = gnp.tile([8, 2, 2], f32, tag=f"mE{half}")
                nc.vector.tensor_scalar_mul(out=mE, in0=pgv[:, sl, :],
                                            scalar1=1.0 / 16.0)
                var_t = gnp.tile([8, 2], f32, tag=f"var{half}")
                nc.vector.tensor_mul(out=var_t, in0=mE[:, :, 0], in1=mE[:, :, 0])
                nc.vector.tensor_sub(out=var_t, in0=mE[:, :, 1], in1=var_t)
                sd = gnp.tile([8, 2], f32, tag=f"sd{half}")
                nc.scalar.activation(out=sd, in_=var_t, func=ACT.Sqrt,
                                     bias=eps_t[0:8, :], scale=1.0)
                m_rs = gnp.tile([8, 2, 2], f32r, tag=f"m_rs{half}")
                nc.vector.tensor_copy(out=m_rs[:, :, 0], in_=mE[:, :, 0])
                with nc.allow_low_precision(reason="f32r rstd for matmul"):
                    nc.vector.reciprocal(out=m_rs[:, :, 1], in_=sd)
                for ci in range(2):
                    ct = 2 * half + ci
                    bc_ps = psB.tile([128, 1024], f32, tag="B", name=f"bc{ct}")
                    nc.tensor.matmul(bc_ps[:, 0:2], lhsT=gbc_r,
                                     rhs=m_rs[:, ci, :], start=True, stop=True)
                    mrt = gnp.tile([128, 2], f32, tag="mrt")
                    nc.vector.tensor_copy(out=mrt, in_=bc_ps[:, 0:2])
                    A_t = gnp.tile([128, 1], f32, tag=f"A{ct}", name=f"A{ct}")
                    nc.vector.tensor_mul(out=A_t, in0=gamma_sb[:, ct:ct + 1],
                                         in1=mrt[:, 1:2])
                    B_t = gnp.tile([128, 1], f32, tag=f"B{ct}", name=f"B{ct}")
                    tmb = gnp.tile([128, 1], f32, tag="tmb")
                    nc.vector.tensor_mul(out=tmb, in0=mrt[:, 0:1], in1=A_t)
                    nc.vector.tensor_sub(out=B_t, in0=beta_sb[:, ct:ct + 1],
                                         in1=tmb)
                    nc.vector.tensor_scalar(
                        out=ht[ct // 2][:, ct % 2, :], in0=xt[ct],
                        scalar1=A_t, scalar2=B_t, op0=ALU.mult, op1=ALU.add)

            # ---------------- QK GEMM (fp8 DR, 2 heads per block) ----------
            # W free layout: q-block b at 256b (rows 0:64 = head 2b,
            # 64:128 = head 2b+1), k-block b at 256b+128; aug block at
            # 2048:2112 (64 rows: tq_h at h, tk_h at 8+h, zeros above).
            tqtkb = cp.tile([16, HW], bf16, tag="tqtkb")

            def qk_block(off, rows, name):
                ps = psA.tile([128, 1024], f32, tag="A", name=name)
                for t in range(2):
                    for n in range(2):
                        nc.tensor.matmul(
                            ps[0:rows, n * 512:(n + 1) * 512],
                            lhsT=wqk[:, t, :, off:off + rows],
                            rhs=ht[t][:, :, n * 512:(n + 1) * 512],
                            start=(t == 0), stop=(t == 1), perf_mode=DR)
                return ps

            def qk_evac(b):
                psq = qk_block(256 * b, 128, f"qkq{b}")
                nc.scalar.copy(out=QsB[0:64, 2 * b, :], in_=psq[0:64, :])
                nc.scalar.copy(out=QsB[64:128, 2 * b + 1, :], in_=psq[64:128, :])
                psk = qk_block(256 * b + 128, 128, f"qkk{b}")
                nc.vector.tensor_copy(out=KsB[0:64, 2 * b, :], in_=psk[0:64, :])
                nc.vector.tensor_copy(out=KsB[64:128, 2 * b + 1, :],
                                      in_=psk[64:128, :])

            qk_evac(0)
            psa_ = qk_block(2048, 64, "qkaug")
            nc.scalar.copy(out=tqtkb, in_=psa_[0:16, :])
            # tq rows -> QsB (64 for even heads, 0 for odd);
            # tk rows -> KsB (65 even, 1 odd)
            nc.sync.dma_start(out=QsBv[64:65, 0], in_=tqtkb[0:8:2, :])
            nc.sync.dma_start(out=QsBv[0:1, 1], in_=tqtkb[1:8:2, :])
            nc.sync.dma_start(out=KsBv[65:66, 0], in_=tqtkb[8:16:2, :])
            nc.sync.dma_start(out=KsBv[1:2, 1], in_=tqtkb[9:16:2, :])
            for b in range(1, 4):
                qk_evac(b)

            # ---------------- V GEMM (fp8 DR) ----------------
            if True:
                for kk in range(NP):
                    ps = psB.tile([128, 1024], f32, tag="B", name=f"v{kk}")
                    for t in range(2):
                        nc.tensor.matmul(
                            ps[:, 0:512],
                            lhsT=ht[t][:, :, kk * 128:(kk + 1) * 128],
                            rhs=wv[:, t, :, :], start=(t == 0), stop=(t == 1),
                            perf_mode=DR)
                    nc.vector.tensor_copy(
                        out=vt[kk // 2][:, kk % 2, :, 0:HD],
                        in_=ps[:, 0:512].rearrange("p (h d) -> p h d", h=NH))

            # ---------------- attention ----------------
            attO = [aop.tile([128, 2, HW], fp8, tag=f"ao{t2}", name=f"ao{t2}")
                    for t2 in range(2)]

            def emit_scores(h):
                pps = []
                for kp in range(4):
                    pps.append(ppp.tile([128, 2, HW], fp8, tag="pp",
                                        name=f"pp{h}_{kp}"))
                for kk in range(NP):
                    ps = psA.tile([128, 1024], f32, tag="A", name=f"sc{h}_{kk}")
                    for n in range(2):
                        nc.tensor.matmul(
                            ps[:, n * 512:(n + 1) * 512],
                            lhsT=KsB[:, h, kk * 128:(kk + 1) * 128],
                            rhs=QsB[:, h, n * 512:(n + 1) * 512],
                            start=True, stop=True)
                    if kk in EXP_DVE_KK:
                        # Schraudolph on DVE, straight to fp8e4m3 bits:
                        # bits = A8*(SCALE*s + cb) + 56 - CC8; f32->uint8
                        # convert saturates, so tiny probs land at 0.
                        nc.vector.tensor_scalar(
                            out=pps[kk // 2][:, kk % 2, :].bitcast(
                                mybir.dt.uint8),
                            in0=ps, scalar1=A8 * SCALE,
                            scalar2=cs8_sb[:, h:h + 1],
                            op0=ALU.mult, op1=ALU.add)
                    else:
                        nc.scalar.activation(
                            out=pps[kk // 2][:, kk % 2, :], in_=ps, func=ACT.Exp,
                            bias=cb8_sb[:, h:h + 1], scale=SCALE)
                return pps

            def emit_av(h, pps):
                pa = psB.tile([128, 1024], f32, tag="B", name=f"pa{h}")
                for kp in range(4):
                    for n in range(2):
                        nc.tensor.matmul(
                            pa[0:HD + 1, n * 512:(n + 1) * 512],
                            lhsT=vt[kp][:, :, h, 0:HD + 1],
                            rhs=pps[kp][:, :, n * 512:(n + 1) * 512],
                            start=(kp == 0), stop=(kp == 3), perf_mode=DR)
                # evacuate unnormalized AV + denominator row to SBUF bf16
                paS = dsbp.tile([65, HW], bf16, tag="paS", name=f"paS{h}")
                with nc.allow_low_precision(reason="bf16 raw attn out"):
                    nc.vector.tensor_copy(out=paS, in_=pa[0:65, :])
                return paS

            def emit_norm(h, paS):
                rbps = psB.tile([128, 1024], f32, tag="B", name=f"rbps{h}")
                for n in range(2):
                    nc.tensor.matmul(
                        rbps[0:64, n * 512:(n + 1) * 512],
                        lhsT=o64b[64:65, :],
                        rhs=paS[64:65, n * 512:(n + 1) * 512],
                        start=True, stop=True)
                rb = rbp.tile([64, HW], f32, tag="rb", name=f"rb{h}")
                with nc.allow_low_precision(reason="recip of softmax denom"):
                    nc.vector.reciprocal_approx_fast(
                        out=rb, in_=rbps[0:64, :])
                t2, jj = h // 4, (h % 4) // 2
                if h % 2 == 1:
                    # odd heads (incl. the last) write attO lower half direct
                    nc.vector.tensor_mul(
                        out=attO[t2][0:64, jj, :], in0=paS[0:64, :], in1=rb)
                else:
                    # even heads land at partitions 64:128 of attO: normalize
                    # into a scratch then partition-shift via on-chip DMA
                    ash = rbp.tile([64, HW], fp8, tag="ash", name=f"ash{h}")
                    nc.vector.tensor_mul(out=ash, in0=paS[0:64, :], in1=rb)
                    nc.sync.dma_start(out=attO[t2][64:128, jj, :], in_=ash)

            prev = emit_scores(0)
            pend = None
            for h in range(NH):
                nxt = emit_scores(h + 1) if h + 1 < NH else None
                cur = emit_av(h, prev)
                if pend is not None:
                    emit_norm(h - 1, pend)
                pend = cur
                prev = nxt
            emit_norm(NH - 1, pend)

            # ---------------- proj + residual ----------------
            for m in range(NT):
                ps = psA.tile([128, 1024], f32, tag="A", name=f"pr{m}")
                for t2 in range(2):
                    for n in range(2):
                        nc.tensor.matmul(
                            ps[:, n * 512:(n + 1) * 512],
                            lhsT=wp[:, t2, :, m * 128:(m + 1) * 128],
                            rhs=attO[t2][:, :, n * 512:(n + 1) * 512],
                            start=(t2 == 0), stop=(t2 == 1), perf_mode=DR)
                nc.vector.scalar_tensor_tensor(
                    out=xt[m], in0=ps, scalar=wtld_sb[:, m:m + 1],
                    in1=xt[m], op0=ALU.add, op1=ALU.add)
                nc.sync.dma_start(out=out_d[m * 128:(m + 1) * 128, :], in_=xt[m])
    nc.compile()
    return nc


def _prep_common(gamma, beta, w_qkv, b_qkv, w_proj, b_proj):
    fp8np = ml_dtypes.float8_e4m3fn
    w_qkv = np.asarray(w_qkv, np.float32)
    w_proj = np.asarray(w_proj, np.float32)
    b_qkv = np.asarray(b_qkv, np.float32)
    b_proj = np.asarray(b_proj, np.float32)
    wq, wk, wvm = w_qkv[0:C], w_qkv[C:2 * C], w_qkv[2 * C:3 * C]
    bq, bk, bv = b_qkv[0:C], b_qkv[C:2 * C], b_qkv[2 * C:3 * C]

    def drpack(WT, m):
        # WT [512 c, m] -> [128 p, 2 t, 2 j, m] with c = 256t + 128j + p
        return np.ascontiguousarray(
            WT.reshape(2, 2, 128, m).transpose(2, 0, 1, 3).astype(fp8np))

    # Wqk free layout: q-block b at 256b (rows 64s+d = head 2b+s chan d),
    # k-block b at 256b+128; aug rows 2048+h = tq_h, 2056+h = tk_h.
    Wqk = np.zeros((2112, C), np.float32)
    for h in range(NH):
        sl = slice(h * HD, (h + 1) * HD)
        b, sp = h // 2, h % 2
        Wqk[256 * b + 64 * sp:256 * b + 64 * sp + 64] = wq[sl]
        Wqk[256 * b + 128 + 64 * sp:256 * b + 128 + 64 * sp + 64] = wk[sl]
        Wqk[2048 + h] = bk[sl] @ wq[sl]
        Wqk[2056 + h] = bq[sl] @ wk[sl]

    # attO channel permutation: c = 256*t2 + 128*j + 64*ph + d holds head
    # h = 4*t2 + 2*j + (1 - ph) channel d (odd heads at lower partitions)
    WpT = w_proj.T  # [512 c_attn, 512 m]
    Wp_perm = np.zeros_like(WpT)
    for c in range(C):
        t2, j, ph, d = c // 256, (c % 256) // 128, (c % 128) // 64, c % 64
        hsrc = 4 * t2 + 2 * j + (1 - ph)
        Wp_perm[c] = WpT[64 * hsrc + d]

    wtld = w_proj @ bv + b_proj

    bqbk = np.array([bq[h * HD:(h + 1) * HD] @ bk[h * HD:(h + 1) * HD]
                     for h in range(NH)], np.float32)
    cb8 = (SCALE * bqbk - OFF).astype(np.float32)
    cs8 = (A8 * (SCALE * bqbk - OFF) + (B8 - CC8)).astype(np.float32)

    return {
        "wqk": drpack(Wqk.T, 2112),
        "wv8": drpack(wvm.T, 512),
        "wp8": drpack(Wp_perm, 512),
        "wtld": np.ascontiguousarray(wtld.astype(np.float32)),
        "gamma": np.ascontiguousarray(np.asarray(gamma, np.float32)),
        "beta": np.ascontiguousarray(np.asarray(beta, np.float32)),
        "gind": np.ascontiguousarray(
            np.repeat(np.eye(8, dtype=np.float32), 16, axis=0)),
        "gbc": np.ascontiguousarray(
            np.repeat(np.eye(8, dtype=np.float32), 16, axis=1)),
        "cb8": cb8,
        "cs8": cs8,
        "onesb": np.ones((1, 8192), ml_dtypes.bfloat16),
        "cnsb": np.concatenate([np.ones((1, 1024), ml_dtypes.bfloat16),
                                np.zeros((1, 1024), ml_dtypes.bfloat16)]),
    }


_NC = None


def kernel(x, gamma, beta, w_qkv, b_qkv, w_proj, b_proj):
    global _NC
    x = np.asarray(x, dtype=np.float32)
    B = x.shape[0]
    assert B == 8
    if _NC is None:
        _NC = _build()
    common = _prep_common(gamma, beta, w_qkv, b_qkv, w_proj, b_proj)
    in_maps = [
        {"x": np.ascontiguousarray(x[b].reshape(C, HW)), **common}
        for b in range(B)
    ]
    res = run_bass_kernel_spmd(_NC, in_maps, core_ids=list(range(8)))
    out = np.stack([res.results[b]["out"] for b in range(B)])
    return out.reshape(B, C, 32, 32).astype(np.float32)
